# revision 17
# baseline (speedup 1.0000x reference)
"""Trainium2 Bass kernel for nn_GraphNeuralNetwork_27728308863842.

The reference model's output is (policy_logits[10000], value[1]) and both
depend ONLY on the global-state path:
    g      = x_global @ glob_w + glob_b                      # [256]
    policy = relu(g @ pol_w1 + pol_b1) @ pol_w2 + pol_b2     # [10000]
    value  = tanh(relu(g @ val_w1 + val_b1) @ val_w2 + val_b2)
The node/edge message-passing loop never feeds the heads (the reference
notes it reproduces that faithfully), so it is dead code and is not
computed here.

Distribution: pol_w2 [256,10000] is column-sharded 8 ways (1250 cols per
core); everything else is replicated. Each core computes the full g/ph
vectors and its policy-logit slice; core 0's value is used.

Layout/perf notes:
- All small weights ride in ONE host-packed [128,1030] f32 buffer (sm):
  one DMA issue instead of eleven (HWDGE issue is ~0.7us each, serialized
  per ring). glob_b is folded into the g matmul as an extra K row.
- pol_b2 is DMA'd straight into the policy PSUM banks; the policy
  matmuls accumulate on top (start=False), so no separate bias add.
- The value is written (tanh) into the same PSUM tile at column 1250, so
  one [1,1251] DMA stores both outputs.
- sm+pb2 issue on the Scalar(ACT) HWDGE ring, the two 640KB pol_w2
  chunks on the Sync ring: issue runs in parallel on both rings.
- A dummy relu early on ACT pulls the 1.3us ACT_TABLE_LOAD off the
  critical path.
"""

import sys

for _p in ("/opt/trn_rl_repo",):
    if _p not in sys.path:
        sys.path.append(_p)

import numpy as np

import concourse.bass as bass
import concourse.bacc as bacc
import concourse.mybir as mybir
from concourse.tile import TileContext
from concourse.bass_utils import run_bass_kernel_spmd

N_CORES = 8
IN_GLOB = 100
HID = 256
POLICY_DIM = 10000
VAL_HID = 128
SHARD = POLICY_DIM // N_CORES  # 1250

F32 = mybir.dt.float32

# Column offsets inside the packed small-weights buffer [128, SM_COLS].
_XG = 0            # [101, 1]   x_global ++ [1.0]
_GW = 1            # [101, 256] glob_w ++ glob_b row
_PW1 = 257         # [128, 512] pol_w1 as two 128-row chunks side by side
_PB1 = 769         # [128, 2]   pol_b1 chunks
_VW1 = 771         # [128, 256] val_w1 as two 128-row chunks
_VB1 = 1027        # [128, 1]
_VW2 = 1028        # [128, 1]
_VB2 = 1029        # [1, 1] at row 0
SM_COLS = 1030

# Filled with the BassKernelResults of the most recent run (for test.py).
LAST_RESULTS = None


def _build_nc():
    nc = bacc.Bacc(
        "TRN2", target_bir_lowering=False, debug=False, num_devices=N_CORES
    )

    sm_d = nc.dram_tensor("sm", [128, SM_COLS], F32, kind="ExternalInput")
    pw2a_d = nc.dram_tensor("pw2a", [128, SHARD], F32, kind="ExternalInput")
    pw2b_d = nc.dram_tensor("pw2b", [128, SHARD], F32, kind="ExternalInput")
    pb2_d = nc.dram_tensor("pb2", [1, SHARD], F32, kind="ExternalInput")
    out_d = nc.dram_tensor("out", [1, SHARD + 1], F32, kind="ExternalOutput")

    ACT = mybir.ActivationFunctionType

    with TileContext(nc) as tc:
        with (
            tc.tile_pool(name="sbuf", bufs=1) as sb,
            tc.tile_pool(name="psum", bufs=4, space=bass.MemorySpace.PSUM) as pp,
            tc.tile_pool(name="psum_pol", bufs=1, space=bass.MemorySpace.PSUM) as pq,
        ):
            # ---- loads on three parallel issue rings -------------------
            # scalar HWDGE: sm in two pieces (xg+gw gate the first matmul)
            # sync HWDGE:   pw2a in three 512-col pieces
            # gpsimd SWDGE: pw2b in three pieces + pb2
            # Fine-grained pw2 pieces let each policy chunk start as soon
            # as its slice lands instead of after the full 640KB.
            CH = [(0, 512), (512, 512), (1024, SHARD - 1024)]
            sm = sb.tile([128, SM_COLS], F32, tag="sm")
            nc.scalar.dma_start(out=sm[:, :_PW1], in_=sm_d[:, :_PW1])
            nc.scalar.dma_start(out=sm[:, _PW1:], in_=sm_d[:, _PW1:])

            ppol = pq.tile([1, 3 * 512], F32, tag="ppol")
            pw2 = [
                sb.tile([128, SHARD], F32, name="pw2_0", tag="pw2_0"),
                sb.tile([128, SHARD], F32, name="pw2_1", tag="pw2_1"),
            ]
            # gpsimd also memsets the PE-warmup tile first (see below)
            warm = sb.tile([128, 512], F32, tag="warm")
            nc.gpsimd.memset(warm[:], 0.0)
            for n0, nn in CH:
                nc.sync.dma_start(
                    out=pw2[0][:, n0:n0 + nn], in_=pw2a_d[:, n0:n0 + nn]
                )
                nc.gpsimd.dma_start(
                    out=pw2[1][:, n0:n0 + nn], in_=pw2b_d[:, n0:n0 + nn]
                )
            pb2 = sb.tile([1, SHARD], F32, tag="pb2")
            nc.gpsimd.dma_start(out=pb2[:], in_=pb2_d[:])

            # ---- warmup (overlaps the DMA phase) -----------------------
            # ACT: pull the 1.3us ACT_TABLE_LOAD off the critical path.
            # PE: dense 512-wide dummy matmuls ramp the HAM clock — cold
            # matmul passes measure ~2-3x slower than warm ones.
            warm2 = sb.tile([128, 1], F32, tag="warm2")
            nc.scalar.activation(warm2[:], warm[:, 0:1], ACT.Relu)
            pwarm = pq.tile([1, 512], F32, tag="pwarm")

            def dummy_mm():
                nc.tensor.matmul(
                    pwarm[:], warm[:, 0:1], warm[:], start=True, stop=True,
                )

            for _ in range(2):
                dummy_mm()

            # ---- g = [x_global;1] @ [glob_w;glob_b], stored transposed -
            gT = sb.tile([128, 2], F32, tag="gT")
            for j in range(2):
                pg = pp.tile([128, 1], F32, tag="acc")
                nc.tensor.matmul(
                    pg[:], sm[0:101, _GW + j * 128:_GW + (j + 1) * 128],
                    sm[0:101, _XG:_XG + 1], start=True, stop=True,
                )
                nc.scalar.activation(gT[:, j:j + 1], pg[:], ACT.Copy)
            dummy_mm()  # keep the PE clock ramped while ACT copies gT

            # ---- ph = relu(g @ pol_w1 + pol_b1), stored transposed -----
            phT = sb.tile([128, 2], F32, tag="phT")
            for j in range(2):
                pph = pp.tile([128, 1], F32, tag="acc")
                for k in range(2):
                    c = _PW1 + k * 256 + j * 128
                    nc.tensor.matmul(
                        pph[:], sm[:, c:c + 128], gT[:, k:k + 1],
                        start=(k == 0), stop=(k == 1),
                    )
                nc.scalar.activation(
                    phT[:, j:j + 1], pph[:], ACT.Relu,
                    bias=sm[:, _PB1 + j:_PB1 + j + 1],
                )
                dummy_mm()  # keep the PE clock ramped while ACT relus phT

            # ---- policy: matmul chunks, bias added on the way out ------
            # DMA can't touch PSUM, so each bank is moved to pol_sb by an
            # otherwise-idle DVE (fused +pol_b2) as soon as it finishes.
            pol_sb = sb.tile([1, SHARD + 1], F32, tag="pol_sb")
            n0 = 0
            while n0 < SHARD:
                nn = min(512, SHARD - n0)
                for k in range(2):
                    nc.tensor.matmul(
                        ppol[:, n0:n0 + nn], phT[:, k:k + 1],
                        pw2[k][:, n0:n0 + nn],
                        start=(k == 0), stop=(k == 1),
                    )
                nc.vector.tensor_add(
                    pol_sb[:, n0:n0 + nn], ppol[:, n0:n0 + nn], pb2[:, n0:n0 + nn]
                )
                n0 += nn

            # ---- value head -> ppol[0, SHARD] --------------------------
            pvh = pp.tile([128, 1], F32, tag="acc")
            for k in range(2):
                c = _VW1 + k * 128
                nc.tensor.matmul(
                    pvh[:], sm[:, c:c + 128], gT[:, k:k + 1],
                    start=(k == 0), stop=(k == 1),
                )
            vhT = sb.tile([VAL_HID, 1], F32, tag="vhT")
            nc.scalar.activation(
                vhT[:], pvh[:VAL_HID, :], ACT.Relu,
                bias=sm[:VAL_HID, _VB1:_VB1 + 1],
            )
            pval = pp.tile([1, 1], F32, tag="acc")
            nc.tensor.matmul(
                pval[:], vhT[:], sm[:VAL_HID, _VW2:_VW2 + 1],
                start=True, stop=True,
            )
            nc.scalar.activation(
                pol_sb[:, SHARD:SHARD + 1], pval[:], ACT.Tanh,
                bias=sm[0:1, _VB2:_VB2 + 1],
            )

            # ---- single store: 1250 logits + value ---------------------
            nc.sync.dma_start(out=out_d[:], in_=pol_sb[:])

    nc.compile()
    return nc


def _in_map_for_core(inputs, core):
    f32 = lambda a: np.asarray(a, dtype=np.float32)
    sm = np.zeros((128, SM_COLS), np.float32)
    sm[0:100, _XG] = f32(inputs["x_global"])
    sm[100, _XG] = 1.0
    sm[0:100, _GW:_GW + HID] = f32(inputs["glob_w"])
    sm[100, _GW:_GW + HID] = f32(inputs["glob_b"])
    pw1 = f32(inputs["pol_w1"])
    sm[:, _PW1:_PW1 + 256] = pw1[0:128]
    sm[:, _PW1 + 256:_PW1 + 512] = pw1[128:256]
    sm[:, _PB1:_PB1 + 2] = f32(inputs["pol_b1"]).reshape(2, 128).T
    vw1 = f32(inputs["val_w1"])
    sm[:, _VW1:_VW1 + 128] = vw1[0:128]
    sm[:, _VW1 + 128:_VW1 + 256] = vw1[128:256]
    sm[0:VAL_HID, _VB1] = f32(inputs["val_b1"])
    sm[0:VAL_HID, _VW2] = f32(inputs["val_w2"]).reshape(VAL_HID)
    sm[0, _VB2] = f32(inputs["val_b2"]).reshape(())

    pw2 = f32(inputs["pol_w2"])[:, core * SHARD:(core + 1) * SHARD]
    return {
        "sm": sm,
        "pw2a": np.ascontiguousarray(pw2[0:128]),
        "pw2b": np.ascontiguousarray(pw2[128:256]),
        "pb2": np.ascontiguousarray(
            f32(inputs["pol_b2"])[core * SHARD:(core + 1) * SHARD].reshape(1, SHARD)
        ),
    }


def kernel(**inputs):
    global LAST_RESULTS
    nc = _build_nc()
    in_maps = [_in_map_for_core(inputs, c) for c in range(N_CORES)]
    res = run_bass_kernel_spmd(nc, in_maps, list(range(N_CORES)))
    LAST_RESULTS = res
    pol = np.concatenate(
        [np.asarray(res.results[c]["out"]).reshape(SHARD + 1)[:SHARD]
         for c in range(N_CORES)]
    ).astype(np.float32)
    val = np.asarray(res.results[0]["out"]).reshape(SHARD + 1)[SHARD:].astype(
        np.float32
    )
    return pol, val


# revision 18
# speedup vs baseline: 1.1217x; 1.1217x over previous
"""Trainium2 Bass kernel for nn_GraphNeuralNetwork_27728308863842.

The reference model's output is (policy_logits[10000], value[1]) and both
depend ONLY on the global-state path:
    g      = x_global @ glob_w + glob_b                      # [256]
    policy = relu(g @ pol_w1 + pol_b1) @ pol_w2 + pol_b2     # [10000]
    value  = tanh(relu(g @ val_w1 + val_b1) @ val_w2 + val_b2)
The node/edge message-passing loop never feeds the heads (the reference
notes it reproduces that faithfully), so it is dead code and is not
computed here.

Distribution: pol_w2 [256,10000] is column-sharded 8 ways (1250 cols per
core); everything else is replicated. Each core computes the full g/ph
vectors and its policy-logit slice; core 0's value is used.

Perf notes (from NTFF traces):
- HWDGE issue is ~0.7us per dma_start and serialized per ring, so loads
  are packed into few fully-contiguous DRAM blocks, split across the two
  HWDGE rings (scalar + sync) in critical-path order.
- glob_b is folded into the g matmul as an extra K row (g is linear).
- The value head runs before the policy phase: it only needs early data,
  and the policy phase is paced by the big pol_w2 transfers.
- A dummy relu early on ACT pulls the 1.3us ACT_TABLE_LOAD off the
  critical path. PE clock-ramp pre-warming does NOT work (ramp decays
  across gaps), so there are no dummy matmuls.
"""

import sys

for _p in ("/opt/trn_rl_repo",):
    if _p not in sys.path:
        sys.path.append(_p)

import numpy as np

import concourse.bass as bass
import concourse.bacc as bacc
import concourse.mybir as mybir
from concourse.tile import TileContext
from concourse.bass_utils import run_bass_kernel_spmd

N_CORES = 8
IN_GLOB = 100
HID = 256
POLICY_DIM = 10000
VAL_HID = 128
SHARD = POLICY_DIM // N_CORES  # 1250

F32 = mybir.dt.float32

# sm1 [128, 257]: col 0 = [x_global;1], cols 1..256 = [glob_w;glob_b]
# sm2 [128, 514]: pol_w1 as two 128-row chunks, then pol_b1 [128,2]
# sm3 [128, 259]: val_w1 as two 128-row chunks, val_b1, val_w2, val_b2@[0]
SM1_COLS = 257
SM2_COLS = 514
SM3_COLS = 259
_VB2_COL = 258

LAST_RESULTS = None


def _build_nc():
    nc = bacc.Bacc(
        "TRN2", target_bir_lowering=False, debug=False, num_devices=N_CORES
    )

    sm1_d = nc.dram_tensor("sm1", [128, SM1_COLS], F32, kind="ExternalInput")
    sm2_d = nc.dram_tensor("sm2", [128, SM2_COLS], F32, kind="ExternalInput")
    sm3_d = nc.dram_tensor("sm3", [128, SM3_COLS], F32, kind="ExternalInput")
    pw2a_d = nc.dram_tensor("pw2a", [128, SHARD], F32, kind="ExternalInput")
    pw2b_d = nc.dram_tensor("pw2b", [128, SHARD], F32, kind="ExternalInput")
    pb2_d = nc.dram_tensor("pb2", [1, SHARD], F32, kind="ExternalInput")
    out_d = nc.dram_tensor("out", [1, SHARD + 1], F32, kind="ExternalOutput")

    ACT = mybir.ActivationFunctionType
    CH = [(0, 512), (512, 512), (1024, SHARD - 1024)]

    with TileContext(nc) as tc:
        with (
            tc.tile_pool(name="sbuf", bufs=1) as sb,
            tc.tile_pool(name="psum", bufs=4, space=bass.MemorySpace.PSUM) as pp,
            tc.tile_pool(name="psum_pol", bufs=1, space=bass.MemorySpace.PSUM) as pq,
        ):
            # ---- loads: critical-path order, two parallel HWDGE rings --
            sm1 = sb.tile([128, SM1_COLS], F32, tag="sm1")
            nc.scalar.dma_start(out=sm1[:], in_=sm1_d[:])
            sm2 = sb.tile([128, SM2_COLS], F32, tag="sm2")
            nc.scalar.dma_start(out=sm2[:], in_=sm2_d[:])
            sm3 = sb.tile([128, SM3_COLS], F32, tag="sm3")
            nc.scalar.dma_start(out=sm3[:], in_=sm3_d[:])

            pw2 = [
                sb.tile([128, SHARD], F32, name="pw2_0", tag="pw2_0"),
                sb.tile([128, SHARD], F32, name="pw2_1", tag="pw2_1"),
            ]
            nc.sync.dma_start(out=pw2[0][:], in_=pw2a_d[:])
            nc.sync.dma_start(out=pw2[1][:], in_=pw2b_d[:])
            pb2 = sb.tile([1, SHARD], F32, tag="pb2")
            nc.sync.dma_start(out=pb2[:], in_=pb2_d[:])

            # ---- ACT table prewarm (overlaps the DMA phase) ------------
            warm = sb.tile([128, 1], F32, tag="warm")
            nc.gpsimd.memset(warm[:], 0.0)
            warm2 = sb.tile([128, 1], F32, tag="warm2")
            nc.scalar.activation(warm2[:], warm[:], ACT.Relu)

            # ---- g = [x_global;1] @ [glob_w;glob_b], stored transposed -
            gT = sb.tile([128, 2], F32, tag="gT")
            for j in range(2):
                pg = pp.tile([128, 1], F32, tag="acc")
                nc.tensor.matmul(
                    pg[:], sm1[0:101, 1 + j * 128:1 + (j + 1) * 128],
                    sm1[0:101, 0:1], start=True, stop=True,
                )
                nc.scalar.activation(gT[:, j:j + 1], pg[:], ACT.Copy)

            # ---- ph = relu(g @ pol_w1 + pol_b1), stored transposed -----
            phT = sb.tile([128, 2], F32, tag="phT")
            for j in range(2):
                pph = pp.tile([128, 1], F32, tag="acc")
                for k in range(2):
                    c = k * 256 + j * 128
                    nc.tensor.matmul(
                        pph[:], sm2[:, c:c + 128], gT[:, k:k + 1],
                        start=(k == 0), stop=(k == 1),
                    )
                nc.scalar.activation(
                    phT[:, j:j + 1], pph[:], ACT.Relu,
                    bias=sm2[:, 512 + j:513 + j],
                )

            # ---- value head (early: doesn't need the big transfers) ----
            pol_sb = sb.tile([1, SHARD + 1], F32, tag="pol_sb")
            pvh = pp.tile([128, 1], F32, tag="acc")
            for k in range(2):
                nc.tensor.matmul(
                    pvh[:], sm3[:, k * 128:(k + 1) * 128], gT[:, k:k + 1],
                    start=(k == 0), stop=(k == 1),
                )
            vhT = sb.tile([VAL_HID, 1], F32, tag="vhT")
            nc.scalar.activation(
                vhT[:], pvh[:VAL_HID, :], ACT.Relu,
                bias=sm3[:VAL_HID, 256:257],
            )
            pval = pp.tile([1, 1], F32, tag="acc")
            nc.tensor.matmul(
                pval[:], vhT[:], sm3[:VAL_HID, 257:258], start=True, stop=True,
            )
            nc.scalar.activation(
                pol_sb[:, SHARD:SHARD + 1], pval[:], ACT.Tanh,
                bias=sm3[0:1, _VB2_COL:_VB2_COL + 1],
            )

            # ---- policy: paced by the pol_w2 transfers -----------------
            ppol = pq.tile([1, 3 * 512], F32, tag="ppol")
            for n0, nn in CH:
                for k in range(2):
                    nc.tensor.matmul(
                        ppol[:, n0:n0 + nn], phT[:, k:k + 1],
                        pw2[k][:, n0:n0 + nn],
                        start=(k == 0), stop=(k == 1),
                    )
                nc.vector.tensor_add(
                    pol_sb[:, n0:n0 + nn], ppol[:, n0:n0 + nn], pb2[:, n0:n0 + nn]
                )

            # ---- single store: 1250 logits + value ---------------------
            nc.sync.dma_start(out=out_d[:], in_=pol_sb[:])

    nc.compile()
    return nc


def _in_map_for_core(inputs, core):
    f32 = lambda a: np.asarray(a, dtype=np.float32)
    sm1 = np.zeros((128, SM1_COLS), np.float32)
    sm1[0:100, 0] = f32(inputs["x_global"])
    sm1[100, 0] = 1.0
    sm1[0:100, 1:257] = f32(inputs["glob_w"])
    sm1[100, 1:257] = f32(inputs["glob_b"])

    sm2 = np.zeros((128, SM2_COLS), np.float32)
    pw1 = f32(inputs["pol_w1"])
    sm2[:, 0:256] = pw1[0:128]
    sm2[:, 256:512] = pw1[128:256]
    sm2[:, 512:514] = f32(inputs["pol_b1"]).reshape(2, 128).T

    sm3 = np.zeros((128, SM3_COLS), np.float32)
    vw1 = f32(inputs["val_w1"])
    sm3[:, 0:128] = vw1[0:128]
    sm3[:, 128:256] = vw1[128:256]
    sm3[0:VAL_HID, 256] = f32(inputs["val_b1"])
    sm3[0:VAL_HID, 257] = f32(inputs["val_w2"]).reshape(VAL_HID)
    sm3[0, _VB2_COL] = f32(inputs["val_b2"]).reshape(())

    pw2 = f32(inputs["pol_w2"])[:, core * SHARD:(core + 1) * SHARD]
    return {
        "sm1": sm1,
        "sm2": sm2,
        "sm3": sm3,
        "pw2a": np.ascontiguousarray(pw2[0:128]),
        "pw2b": np.ascontiguousarray(pw2[128:256]),
        "pb2": np.ascontiguousarray(
            f32(inputs["pol_b2"])[core * SHARD:(core + 1) * SHARD].reshape(1, SHARD)
        ),
    }


def kernel(**inputs):
    global LAST_RESULTS
    nc = _build_nc()
    in_maps = [_in_map_for_core(inputs, c) for c in range(N_CORES)]
    res = run_bass_kernel_spmd(nc, in_maps, list(range(N_CORES)))
    LAST_RESULTS = res
    pol = np.concatenate(
        [np.asarray(res.results[c]["out"]).reshape(SHARD + 1)[:SHARD]
         for c in range(N_CORES)]
    ).astype(np.float32)
    val = np.asarray(res.results[0]["out"]).reshape(SHARD + 1)[SHARD:].astype(
        np.float32
    )
    return pol, val
